# revision 1
# baseline (speedup 1.0000x reference)
"""TSM-style 3-tap depthwise temporal conv on 8 Trainium2 NeuronCores.

out[n, t, c, h, w] = w[c,0]*x[n,t-1,c,h,w] + w[c,1]*x[n,t,c,h,w]
                   + w[c,2]*x[n,t+1,c,h,w]   (zero-padded at clip edges)

Sharding: pure data parallel over the nt (clip-batch) axis — each of the 8
cores gets whole clips (nt=64, n_segment=8 -> one 8-frame clip per core).
Weight (c,3) is replicated.

This platform has a large fixed cost per *instruction* (measured ~60-100us
on the compute engines, independent of operand size, with DMA transfers
comparatively cheap), so the kernel minimizes instruction count: per
channel-block of 128 channels it loads the whole clip into one SBUF tile
(128 x 8 x 3136), applies the 3-tap conv as three full-clip fused ops on
three different engines, and stores with one DMA:

  DVE:  y          = x * w1                (tensor_scalar_mul)
  DVE:  y[:, 1:]  += x[:, :-1] * w0        (scalar_tensor_tensor)
  DVE:  y[:, :-1] += x[:, 1:]  * w2        (scalar_tensor_tensor)

10 instructions per core per pass (2 loads + 6 DVE ops + 2 stores) instead
of the naive ~76. Measured on this platform: instructions serialize
globally (~90us per compute op, ~30us per DMA), so total time tracks
instruction count; ACT ops cost ~1.5x DVE ops, hence all-DVE compute.
"""

import numpy as np

import concourse.bacc as bacc
import concourse.mybir as mybir
import concourse.tile as tile
from concourse.bass_utils import run_bass_kernel_spmd

N_CORES = 8
P = 128  # SBUF partitions

_cache = {}


def _emit_conv(nc, tc, pools, src, dst, wt_by_blk, F, C, HW, n_seg, uid,
               shift_engine="vector", mul_engine="vector",
               load_engines=("gpsimd",), store_engines=("gpsimd",),
               hw_split=1):
    """Emit one full conv pass src -> dst (both DRAM (F, C, HW) handles).

    hw_split > 1 tiles the hw axis (shift ops never cross hw, so no seams);
    smaller tiles allow bufs >= 2 for cross-block pipelining.
    """
    wp, xp, yp = pools
    mult = mybir.AluOpType.mult
    add = mybir.AluOpType.add
    nblk = C // P
    n_clips = max(F // n_seg, 1)
    S = min(n_seg, F)
    eng2 = getattr(nc, shift_engine)
    HWs = HW // hw_split

    def split_dma(engines, sbuf_tile, dram_view, is_load):
        n = len(engines)
        step = F // n
        for i, ename in enumerate(engines):
            eng = getattr(nc, ename)
            fs = slice(i * step, (i + 1) * step if i < n - 1 else F)
            if is_load:
                eng.dma_start(out=sbuf_tile[:, fs, :], in_=dram_view[:, fs, :])
            else:
                eng.dma_start(out=dram_view[:, fs, :], in_=sbuf_tile[:, fs, :])

    for b in range(nblk):
        cs = slice(b * P, (b + 1) * P)
        wt = wt_by_blk[b]
        w0, w1, w2 = wt[:, 0:1], wt[:, 1:2], wt[:, 2:3]

        for h in range(hw_split):
            hs = slice(h * HWs, (h + 1) * HWs)
            xt = xp.tile([P, F, HWs], mybir.dt.float32, tag="x",
                         name=f"x{uid}_{b}_{h}")
            src_v = src[:, cs, hs].rearrange("f c x -> c f x")
            split_dma(load_engines, xt, src_v, True)

            y = yp.tile([P, F, HWs], mybir.dt.float32, tag="y",
                        name=f"y{uid}_{b}_{h}")
            if mul_engine == "scalar":
                nc.scalar.mul(y[:], xt[:], w1)
            else:
                nc.vector.tensor_scalar_mul(y[:], xt[:], w1)
            for c in range(n_clips):
                lo, hi = c * S, (c + 1) * S
                nc.vector.scalar_tensor_tensor(
                    y[:, lo + 1 : hi, :], xt[:, lo : hi - 1, :], w0,
                    y[:, lo + 1 : hi, :], mult, add,
                )
                eng2.scalar_tensor_tensor(
                    y[:, lo : hi - 1, :], xt[:, lo + 1 : hi, :], w2,
                    y[:, lo : hi - 1, :], mult, add,
                )
            dst_v = dst[:, cs, hs].rearrange("f c x -> c f x")
            split_dma(store_engines, y, dst_v, False)


def _build(F, C, HW, n_seg, repeat=1, x_bufs=1, y_bufs=1):
    """One-core program: x (F, C, HW) -> out (F, C, HW).

    repeat > 1 chains the conv through internal DRAM ping-pong buffers —
    identical HBM traffic per pass; used by the timing harness.
    """
    nc = bacc.Bacc(
        "TRN2",
        target_bir_lowering=False,
        debug=False,
        num_devices=N_CORES,
    )
    x = nc.dram_tensor("x", (F, C, HW), mybir.dt.float32, kind="ExternalInput")
    w = nc.dram_tensor("weight", (C, 3), mybir.dt.float32, kind="ExternalInput")
    out = nc.dram_tensor("out", (F, C, HW), mybir.dt.float32, kind="ExternalOutput")
    scratch = [
        nc.dram_tensor(f"scratch{i}", (F, C, HW), mybir.dt.float32, kind="Internal")
        for i in range(2 if repeat > 1 else 0)
    ]

    nblk = C // P
    with tile.TileContext(nc) as tc:
        with (
            tc.tile_pool(name="wp", bufs=1) as wp,
            tc.tile_pool(name="xp", bufs=x_bufs) as xp,
            tc.tile_pool(name="yp", bufs=y_bufs) as yp,
        ):
            # all channel-blocks' weights in one DMA: partition p holds
            # channels p, p+128, ... as (nblk, 3) in the free dim
            wtile = wp.tile([P, nblk, 3], mybir.dt.float32, tag="w", name="wtile")
            nc.sync.dma_start(
                out=wtile[:], in_=w.ap().rearrange("(b c) k -> c b k", c=P)
            )
            wt_by_blk = [wtile[:, b, :] for b in range(nblk)]

            pools = (wp, xp, yp)
            for k in range(repeat):
                src = x if k == 0 else scratch[k % 2]
                dst = out if k == repeat - 1 else scratch[(k + 1) % 2]
                _emit_conv(nc, tc, pools, src, dst, wt_by_blk, F, C, HW, n_seg, k)
    nc.compile()
    return nc


def _get_program(F, C, HW, n_seg, repeat=1):
    key = (F, C, HW, n_seg, repeat)
    if key not in _cache:
        _cache[key] = _build(F, C, HW, n_seg, repeat=repeat)
    return _cache[key]


def kernel(x, weight, n_segment, **_kw):
    x = np.asarray(x)
    weight = np.ascontiguousarray(np.asarray(weight, dtype=np.float32))
    n_seg = int(np.asarray(n_segment))
    nt, C, H, W = x.shape
    HW = H * W
    assert nt % N_CORES == 0
    F = nt // N_CORES
    # each core must hold whole clips
    assert F % n_seg == 0 or n_seg % F == 0, (F, n_seg)

    nc = _get_program(F, C, HW, n_seg)

    xs = np.ascontiguousarray(x, dtype=np.float32).reshape(nt, C, HW)
    in_maps = [
        {"x": xs[i * F : (i + 1) * F], "weight": weight} for i in range(N_CORES)
    ]
    res = run_bass_kernel_spmd(nc, in_maps, list(range(N_CORES)))
    out = np.concatenate([res.results[i]["out"] for i in range(N_CORES)], axis=0)
    return out.reshape(nt, C, H, W).astype(x.dtype, copy=False)

